# revision 6
# baseline (speedup 1.0000x reference)
"""Trainium2 Bass kernel for nn_Attend (l2-dist attention, b=4 h=8 n=2048 d=64).

Reference math:
    sim = 2*scale*(q@k^T) - ||q||^2 - ||k||^2   (scale = d^-0.5)
    sim = where(mask_j, sim, -FLT_MAX)
    out = softmax_j(sim) @ v

Device strategy (8 cores, pure data/head parallel, no collectives):
  - (b, h) pairs flattened; core c handles b = c//2, heads 4*(c%2)..+4.
  - ||q||^2 is constant per softmax row -> dropped (softmax shift-invariant).
  - mask is per (b, j): ~50% of keys masked.  Host compacts k/v to the valid
    columns only (padded to a multiple of 128), halving all device work.
  - No row-max pass: logits + C stay comfortably inside fp32 exp range.
    Per-key bias (C - ||k_j||^2, or -1e30 for padding) is applied via the
    ACT engine's per-partition bias during the exp.
  - Layout: S^T = K_c @ Q^T computed with keys on partitions (fp16 matmul,
    fp32 PSUM accumulate), exp on ACT -> P^T in fp16, then out^acc[i,65] +=
    P^T_tile^T @ [V|1] (fp16) accumulated over key tiles in PSUM.  Column 64
    (the ones column) is the softmax denominator; DVE reciprocal+scale
    finishes the division.  Host pre-transposes/casts/pads everything so the
    device does zero layout work.
"""

import os
import sys

import numpy as np

for _p in ("/root/.axon_site/_ro/trn_rl_repo", "/opt/trn_rl_repo"):
    if os.path.isdir(_p) and _p not in sys.path:
        sys.path.append(_p)

from contextlib import ExitStack

import concourse.bacc as bacc
import concourse.tile as tile
from concourse import mybir
from concourse.bass_utils import run_bass_kernel_spmd

N_CORES = 8
N_I = 2048          # queries per head
D = 64
HEADS_PER_CORE = 4
C_SHIFT = 30.0      # logit shift; keeps exp inputs in a comfortable range
PAD_BIAS = -1e30    # exp() underflows to exactly 0

_PROGRAM_CACHE = {}


def _build_program(j_tiles: int):
    """Bass program for one core: 4 heads of compacted attention."""
    nc = bacc.Bacc("TRN2", target_bir_lowering=False, debug=False)
    jp = j_tiles * 128
    f16, f32 = mybir.dt.float16, mybir.dt.float32

    qT = nc.dram_tensor("qT", [2, 128, N_I], f16, kind="ExternalInput").ap()
    kT = nc.dram_tensor("kT", [2, 128, jp], f16, kind="ExternalInput").ap()
    vS = nc.dram_tensor("vS", [4, 128, j_tiles * 65], f16, kind="ExternalInput").ap()
    bias = nc.dram_tensor("bias", [4, 128, j_tiles], f32, kind="ExternalInput").ap()
    out = nc.dram_tensor("out", [4, 128, 16, 64], f32, kind="ExternalOutput").ap()

    with tile.TileContext(nc) as tc, ExitStack() as ctx:
        inp = ctx.enter_context(tc.tile_pool(name="inp", bufs=1))
        pp = ctx.enter_context(tc.tile_pool(name="pp", bufs=3))
        outp = ctx.enter_context(tc.tile_pool(name="outp", bufs=2))
        rp = ctx.enter_context(tc.tile_pool(name="rp", bufs=2))
        ps_st = ctx.enter_context(tc.tile_pool(name="ps_st", bufs=2, space="PSUM"))
        ps_acc = ctx.enter_context(tc.tile_pool(name="ps_acc", bufs=4, space="PSUM"))

        qT_t, kT_t = [], []
        for t in range(2):
            qt = inp.tile([128, N_I], f16, tag=f"q{t}")
            nc.sync.dma_start(qt[:], qT[t])
            qT_t.append(qt)
            kt = inp.tile([128, jp], f16, tag=f"k{t}")
            nc.sync.dma_start(kt[:], kT[t])
            kT_t.append(kt)
        vS_t, bias_t = [], []
        for hh in range(4):
            vt = inp.tile([128, j_tiles * 65], f16, tag=f"v{hh}")
            nc.sync.dma_start(vt[:], vS[hh])
            vS_t.append(vt)
            bt = inp.tile([128, j_tiles], f32, tag=f"b{hh}")
            nc.sync.dma_start(bt[:], bias[hh])
            bias_t.append(bt)

        # Flat software pipeline over stages (hh, ih, jt): QK for stage s+1 is
        # emitted BEFORE the PV of stage s so the PE computes the next S^T
        # while ACT runs the current exp — otherwise each stage serializes
        # ACT -> PV -> QK -> ACT.
        stages = [
            (hh, ih, jt)
            for hh in range(HEADS_PER_CORE)
            for ih in range(2)
            for jt in range(j_tiles)
        ]
        st_tiles = {}
        acc_tiles = {}
        osb_tiles = {}

        def emit_qk(s):
            hh, ih, jt = stages[s]
            th, ph = hh // 2, 64 * (hh % 2)
            st = ps_st.tile([128, 1024], f32, tag="st", name=f"st_{hh}_{ih}_{jt}")
            for half in range(2):
                i0 = ih * 1024 + half * 512
                nc.tensor.matmul(
                    st[:, half * 512:(half + 1) * 512],
                    kT_t[th][ph:ph + 64, jt * 128:(jt + 1) * 128],
                    qT_t[th][ph:ph + 64, i0:i0 + 512],
                    start=True, stop=True,
                )
            st_tiles[s] = st

        emit_qk(0)
        for s, (hh, ih, jt) in enumerate(stages):
            st = st_tiles.pop(s)
            pt = pp.tile([128, 1024], f16, tag="pt", name=f"pt_{hh}_{ih}_{jt}")
            nc.scalar.activation(
                pt[:], st[:], mybir.ActivationFunctionType.Exp,
                bias=bias_t[hh][:, jt:jt + 1], scale=1.0,
            )
            if s + 1 < len(stages):
                emit_qk(s + 1)
            if jt == 0:
                acc_tiles[(hh, ih)] = [
                    ps_acc.tile([128, 4, 65], f32, tag="acc", name=f"acc_{hh}_{ih}_{g}")
                    for g in range(2)
                ]
            accs = acc_tiles[(hh, ih)]
            for sl in range(8):
                # start=True lazily zeroes the WHOLE 2KB psum bank (pending-
                # zero bits); only the first slice-matmul of each bank may
                # carry it.  Later slices at jt==0 then overwrite their
                # still-pending bytes.
                nc.tensor.matmul(
                    accs[sl // 4][:, sl % 4, :],
                    pt[:, sl * 128:(sl + 1) * 128],
                    vS_t[hh][:, jt * 65:(jt + 1) * 65],
                    start=(jt == 0 and sl % 4 == 0),
                    stop=(jt == j_tiles - 1 and sl % 4 == 3),
                    skip_group_check=True,
                )
            if jt == j_tiles - 1:
                if hh not in osb_tiles:
                    osb_tiles[hh] = outp.tile(
                        [128, 16, 64], f32, tag="osb", name=f"osb_{hh}"
                    )
                osb = osb_tiles[hh]
                for g in range(2):
                    r = rp.tile([128, 4], f32, tag="r", name=f"r_{hh}_{ih}_{g}")
                    nc.vector.reciprocal(r[:], accs[g][:, :, 64])
                    for sl in range(4):
                        t_idx = ih * 8 + g * 4 + sl
                        nc.vector.tensor_scalar_mul(
                            osb[:, t_idx, :], accs[g][:, sl, 0:64], r[:, sl:sl + 1]
                        )
                del acc_tiles[(hh, ih)]
                if ih == 1:
                    nc.sync.dma_start(out[hh], osb[:])
                    del osb_tiles[hh]

    nc.compile()
    return nc


def _get_program(j_tiles: int):
    if j_tiles not in _PROGRAM_CACHE:
        _PROGRAM_CACHE[j_tiles] = _build_program(j_tiles)
    return _PROGRAM_CACHE[j_tiles]


def _prepare_inputs(q, k, v, mask, j_tiles, idxs):
    """Host-side shard + compact + transpose + cast for each core."""
    b, h, n, d = q.shape
    scale = d ** -0.5
    jp = j_tiles * 128
    in_maps = []
    for c in range(N_CORES):
        bi = c // 2
        ix = idxs[bi]
        nv = len(ix)
        qT_np = np.zeros((2, 128, N_I), np.float16)
        kT_np = np.zeros((2, 128, jp), np.float16)
        vS_np = np.zeros((4, 128, j_tiles * 65), np.float16)
        bias_np = np.full((4, 128, j_tiles), PAD_BIAS, np.float32)
        for hh in range(4):
            hi = (c % 2) * 4 + hh
            th, ph = hh // 2, 64 * (hh % 2)
            qT_np[th, ph:ph + 64, :] = (2.0 * scale * q[bi, hi]).T.astype(np.float16)
            kc = k[bi, hi, ix, :]
            kT_np[th, ph:ph + 64, :nv] = kc.T.astype(np.float16)
            vc = v[bi, hi, ix, :]
            va = np.concatenate(
                [vc, np.ones((nv, 1), np.float32)], axis=1
            ).astype(np.float16)
            vfull = np.zeros((jp, 65), np.float16)
            vfull[:nv] = va
            vS_np[hh] = (
                vfull.reshape(j_tiles, 128, 65).transpose(1, 0, 2)
                .reshape(128, j_tiles * 65)
            )
            ksq = (kc.astype(np.float64) ** 2).sum(-1).astype(np.float32)
            bfull = np.full((jp,), PAD_BIAS, np.float32)
            bfull[:nv] = C_SHIFT - ksq
            bias_np[hh] = bfull.reshape(j_tiles, 128).T
        in_maps.append({"qT": qT_np, "kT": kT_np, "vS": vS_np, "bias": bias_np})
    return in_maps


def _install_profile_shim():
    """Bridge concourse's NTFF trace path to the in-container profiler.

    concourse expects `antenv.axon_hooks.{get,set}_axon_ntff_profile_hook`;
    this image's antenv stub lacks it.  Recreate the module and register the
    ctypes hook from trn_agent_boot.  Also neuter upload_artifacts (no cloud
    bucket in-container).
    """
    import types
    import contextlib

    try:
        import antenv
        if "antenv.axon_hooks" not in sys.modules:
            mod = types.ModuleType("antenv.axon_hooks")
            mod._hook = None

            def set_axon_ntff_profile_hook(h):
                mod._hook = h

            def get_axon_ntff_profile_hook():
                return mod._hook

            mod.set_axon_ntff_profile_hook = set_axon_ntff_profile_hook
            mod.get_axon_ntff_profile_hook = get_axon_ntff_profile_hook
            sys.modules["antenv.axon_hooks"] = mod
            antenv.axon_hooks = mod
        from antenv import axon_hooks
        if axon_hooks.get_axon_ntff_profile_hook() is None:
            from trn_agent_boot.trn_boot import _ntff_profile_via_ctypes
            axon_hooks.set_axon_ntff_profile_hook(
                _ntff_profile_via_ctypes("/opt/axon/libaxon_pjrt.so")
            )
        import concourse.bass_utils as bu
        bu.upload_artifacts = lambda d: str(d)
        return axon_hooks.get_axon_ntff_profile_hook() is not None
    except Exception as e:  # pragma: no cover - profiling is best-effort
        print(f"profile shim failed: {e}")
        return False


def kernel(q, k, v, mask, _profile=False, _trace_kwargs=None):
    q = np.asarray(q, dtype=np.float32)
    k = np.asarray(k, dtype=np.float32)
    v = np.asarray(v, dtype=np.float32)
    mask = np.asarray(mask)
    b, h, n, d = q.shape

    idxs = [np.nonzero(mask[bi])[0] for bi in range(b)]
    max_nv = max(max(len(ix) for ix in idxs), 1)
    j_tiles = -(-max_nv // 128)

    nc = _get_program(j_tiles)
    in_maps = _prepare_inputs(q, k, v, mask, j_tiles, idxs)

    kwargs = {}
    if _profile and _install_profile_shim():
        kwargs["trace"] = True
        if _trace_kwargs:
            kwargs["trace_kwargs"] = _trace_kwargs
    res = run_bass_kernel_spmd(nc, in_maps, list(range(N_CORES)), **kwargs)

    out = np.empty((b, h, n, d), np.float32)
    for c in range(N_CORES):
        o = res.results[c]["out"]  # [4, 128, 16, 64]
        bi = c // 2
        for hh in range(4):
            hi = (c % 2) * 4 + hh
            out[bi, hi] = o[hh].transpose(1, 0, 2).reshape(n, d)
    if _profile:
        return out, res
    return out


# revision 7
# speedup vs baseline: 1.0591x; 1.0591x over previous
"""Trainium2 Bass kernel for nn_Attend (l2-dist attention, b=4 h=8 n=2048 d=64).

Reference math:
    sim = 2*scale*(q@k^T) - ||q||^2 - ||k||^2   (scale = d^-0.5)
    sim = where(mask_j, sim, -FLT_MAX)
    out = softmax_j(sim) @ v

Device strategy (8 cores, pure data/head parallel, no collectives):
  - (b, h) pairs flattened; core c handles b = c//2, heads 4*(c%2)..+4.
  - ||q||^2 is constant per softmax row -> dropped (softmax shift-invariant).
  - mask is per (b, j): ~50% of keys masked.  Host compacts k/v to the valid
    columns only (padded to a multiple of 128), halving all device work.
  - No row-max pass: logits + C stay comfortably inside fp32 exp range.
    Per-key bias (C - ||k_j||^2, or -1e30 for padding) is applied via the
    ACT engine's per-partition bias during the exp.
  - Layout: S^T = K_c @ Q^T with keys on partitions (fp16 matmul, fp32 PSUM),
    exp on ACT -> P^T in fp16, then acc[i, 65] += P^T_slice^T @ [V|1] (fp16)
    accumulated over key tiles in PSUM.  Column 64 (ones) is the softmax
    denominator; DVE reciprocal + per-partition scale finishes the division.
  - q/k are duplicated into both partition halves so the two K=64 QK matmuls
    of a stage run CONCURRENTLY in different PE row-groups (row tiling);
    stages are software-pipelined so the next S^T computes during the
    current exp.  Host pre-transposes/casts/pads; device does no layout work.
"""

import os
import sys

import numpy as np

for _p in ("/root/.axon_site/_ro/trn_rl_repo", "/opt/trn_rl_repo"):
    if os.path.isdir(_p) and _p not in sys.path:
        sys.path.append(_p)

from contextlib import ExitStack

import concourse.bacc as bacc
import concourse.tile as tile
from concourse import mybir
from concourse.bass_utils import run_bass_kernel_spmd

N_CORES = 8
N_I = 2048          # queries per head
D = 64
HEADS_PER_CORE = 4
C_SHIFT = 30.0      # logit shift; keeps exp inputs in a comfortable range
PAD_BIAS = -1e30    # exp() underflows to exactly 0

_PROGRAM_CACHE = {}


def _build_program(j_tiles: int):
    """Bass program for one core: 4 heads of compacted attention."""
    nc = bacc.Bacc("TRN2", target_bir_lowering=False, debug=False)
    jp = j_tiles * 128
    f16, f32 = mybir.dt.float16, mybir.dt.float32

    # DRAM layouts mirror SBUF exactly: [128 partitions, ...] contiguous.
    qT = nc.dram_tensor("qT", [4, 128, N_I], f16, kind="ExternalInput").ap()
    kT = nc.dram_tensor("kT", [4, 128, jp], f16, kind="ExternalInput").ap()
    vS = nc.dram_tensor("vS", [4, 128, j_tiles * 65], f16, kind="ExternalInput").ap()
    bias = nc.dram_tensor("bias", [4, 128, j_tiles], f32, kind="ExternalInput").ap()
    out = nc.dram_tensor("out", [4, 128, 16, 64], f32, kind="ExternalOutput").ap()

    with tile.TileContext(nc) as tc, ExitStack() as ctx:
        inp = ctx.enter_context(tc.tile_pool(name="inp", bufs=1))
        pp = ctx.enter_context(tc.tile_pool(name="pp", bufs=3))
        outp = ctx.enter_context(tc.tile_pool(name="outp", bufs=2))
        rp = ctx.enter_context(tc.tile_pool(name="rp", bufs=2))
        ps_st = ctx.enter_context(tc.tile_pool(name="ps_st", bufs=2, space="PSUM"))
        ps_acc = ctx.enter_context(tc.tile_pool(name="ps_acc", bufs=4, space="PSUM"))

        # Per-head input tiles, DMA'd in head order so head 0 compute starts
        # as early as possible.
        qT_t, kT_t, vS_t, bias_t = [], [], [], []
        for hh in range(HEADS_PER_CORE):
            qt = inp.tile([128, N_I], f16, tag=f"q{hh}", name=f"qt{hh}")
            nc.sync.dma_start(qt[:], qT[hh])
            qT_t.append(qt)
            kt = inp.tile([128, jp], f16, tag=f"k{hh}", name=f"kt{hh}")
            nc.sync.dma_start(kt[:], kT[hh])
            kT_t.append(kt)
            bt = inp.tile([128, j_tiles], f32, tag=f"b{hh}", name=f"bt{hh}")
            nc.sync.dma_start(bt[:], bias[hh])
            bias_t.append(bt)
            vt = inp.tile([128, j_tiles * 65], f16, tag=f"v{hh}", name=f"vt{hh}")
            nc.sync.dma_start(vt[:], vS[hh])
            vS_t.append(vt)

        # Flat software pipeline over stages (hh, ih, jt): QK for stage s+1 is
        # emitted BEFORE the PV of stage s so the PE computes the next S^T
        # while ACT runs the current exp.
        stages = [
            (hh, ih, jt)
            for hh in range(HEADS_PER_CORE)
            for ih in range(2)
            for jt in range(j_tiles)
        ]
        st_tiles = {}
        acc_tiles = {}
        osb_tiles = {}

        def emit_qk(s):
            hh, ih, jt = stages[s]
            st = ps_st.tile([128, 1024], f32, tag="st", name=f"st_{hh}_{ih}_{jt}")
            # q/k live duplicated in both partition halves: the two K=64
            # matmuls go to PE row-groups 0 and 64 and run concurrently.
            for half in range(2):
                i0 = ih * 1024 + half * 512
                p0 = 64 * half
                nc.tensor.matmul(
                    st[:, half * 512:(half + 1) * 512],
                    kT_t[hh][p0:p0 + 64, jt * 128:(jt + 1) * 128],
                    qT_t[hh][p0:p0 + 64, i0:i0 + 512],
                    start=True, stop=True,
                )
            st_tiles[s] = st

        emit_qk(0)
        for s, (hh, ih, jt) in enumerate(stages):
            st = st_tiles.pop(s)
            pt = pp.tile([128, 1024], f16, tag="pt", name=f"pt_{hh}_{ih}_{jt}")
            nc.scalar.activation(
                pt[:], st[:], mybir.ActivationFunctionType.Exp,
                bias=bias_t[hh][:, jt:jt + 1], scale=1.0,
            )
            if s + 1 < len(stages):
                emit_qk(s + 1)
            if jt == 0:
                acc_tiles[(hh, ih)] = [
                    ps_acc.tile([128, 4, 65], f32, tag="acc", name=f"acc_{hh}_{ih}_{g}")
                    for g in range(2)
                ]
            accs = acc_tiles[(hh, ih)]
            for sl in range(8):
                # start=True lazily zeroes the WHOLE 2KB psum bank (pending-
                # zero bits); only the first slice-matmul of each bank may
                # carry it.  Later slices at jt==0 then overwrite their
                # still-pending bytes.
                nc.tensor.matmul(
                    accs[sl // 4][:, sl % 4, :],
                    pt[:, sl * 128:(sl + 1) * 128],
                    vS_t[hh][:, jt * 65:(jt + 1) * 65],
                    start=(jt == 0 and sl % 4 == 0),
                    stop=(jt == j_tiles - 1 and sl % 4 == 3),
                    skip_group_check=True,
                )
            if jt == j_tiles - 1:
                if hh not in osb_tiles:
                    osb_tiles[hh] = outp.tile(
                        [128, 16, 64], f32, tag="osb", name=f"osb_{hh}"
                    )
                osb = osb_tiles[hh]
                for g in range(2):
                    r = rp.tile([128, 4], f32, tag="r", name=f"r_{hh}_{ih}_{g}")
                    nc.vector.reciprocal(r[:], accs[g][:, :, 64])
                    for sl in range(4):
                        t_idx = ih * 8 + g * 4 + sl
                        nc.vector.tensor_scalar_mul(
                            osb[:, t_idx, :], accs[g][:, sl, 0:64], r[:, sl:sl + 1]
                        )
                del acc_tiles[(hh, ih)]
                if ih == 1:
                    nc.sync.dma_start(out[hh], osb[:])
                    del osb_tiles[hh]

    nc.compile()
    return nc


def _get_program(j_tiles: int):
    if j_tiles not in _PROGRAM_CACHE:
        _PROGRAM_CACHE[j_tiles] = _build_program(j_tiles)
    return _PROGRAM_CACHE[j_tiles]


def _prepare_inputs(q, k, v, mask, j_tiles, idxs):
    """Host-side shard + compact + transpose + cast for each core."""
    b, h, n, d = q.shape
    scale = d ** -0.5
    jp = j_tiles * 128
    in_maps = []
    for c in range(N_CORES):
        bi = c // 2
        ix = idxs[bi]
        nv = len(ix)
        qT_np = np.zeros((4, 128, N_I), np.float16)
        kT_np = np.zeros((4, 128, jp), np.float16)
        vS_np = np.zeros((4, 128, j_tiles * 65), np.float16)
        bias_np = np.full((4, 128, j_tiles), PAD_BIAS, np.float32)
        for hh in range(4):
            hi = (c % 2) * 4 + hh
            qt = (2.0 * scale * q[bi, hi]).T.astype(np.float16)   # [64, 2048]
            qT_np[hh, 0:64, :] = qt
            qT_np[hh, 64:128, :] = qt
            kc = k[bi, hi, ix, :]
            kt = np.zeros((64, jp), np.float16)
            kt[:, :nv] = kc.T.astype(np.float16)
            kT_np[hh, 0:64, :] = kt
            kT_np[hh, 64:128, :] = kt
            vc = v[bi, hi, ix, :]
            va = np.concatenate(
                [vc, np.ones((nv, 1), np.float32)], axis=1
            ).astype(np.float16)
            vfull = np.zeros((jp, 65), np.float16)
            vfull[:nv] = va
            vS_np[hh] = (
                vfull.reshape(j_tiles, 128, 65).transpose(1, 0, 2)
                .reshape(128, j_tiles * 65)
            )
            ksq = (kc.astype(np.float64) ** 2).sum(-1).astype(np.float32)
            bfull = np.full((jp,), PAD_BIAS, np.float32)
            bfull[:nv] = C_SHIFT - ksq
            bias_np[hh] = bfull.reshape(j_tiles, 128).T
        in_maps.append({"qT": qT_np, "kT": kT_np, "vS": vS_np, "bias": bias_np})
    return in_maps


def _install_profile_shim():
    """Bridge concourse's NTFF trace path to the in-container profiler.

    concourse expects `antenv.axon_hooks.{get,set}_axon_ntff_profile_hook`;
    this image's antenv stub lacks it.  Recreate the module and register the
    ctypes hook from trn_agent_boot.  Also neuter upload_artifacts (no cloud
    bucket in-container).
    """
    import types

    try:
        import antenv
        if "antenv.axon_hooks" not in sys.modules:
            mod = types.ModuleType("antenv.axon_hooks")
            mod._hook = None

            def set_axon_ntff_profile_hook(h):
                mod._hook = h

            def get_axon_ntff_profile_hook():
                return mod._hook

            mod.set_axon_ntff_profile_hook = set_axon_ntff_profile_hook
            mod.get_axon_ntff_profile_hook = get_axon_ntff_profile_hook
            sys.modules["antenv.axon_hooks"] = mod
            antenv.axon_hooks = mod
        from antenv import axon_hooks
        if axon_hooks.get_axon_ntff_profile_hook() is None:
            from trn_agent_boot.trn_boot import _ntff_profile_via_ctypes
            axon_hooks.set_axon_ntff_profile_hook(
                _ntff_profile_via_ctypes("/opt/axon/libaxon_pjrt.so")
            )
        import concourse.bass_utils as bu
        bu.upload_artifacts = lambda d: str(d)
        return axon_hooks.get_axon_ntff_profile_hook() is not None
    except Exception as e:  # pragma: no cover - profiling is best-effort
        print(f"profile shim failed: {e}")
        return False


def kernel(q, k, v, mask, _profile=False, _trace_kwargs=None):
    q = np.asarray(q, dtype=np.float32)
    k = np.asarray(k, dtype=np.float32)
    v = np.asarray(v, dtype=np.float32)
    mask = np.asarray(mask)
    b, h, n, d = q.shape

    idxs = [np.nonzero(mask[bi])[0] for bi in range(b)]
    max_nv = max(max(len(ix) for ix in idxs), 1)
    j_tiles = -(-max_nv // 128)

    nc = _get_program(j_tiles)
    in_maps = _prepare_inputs(q, k, v, mask, j_tiles, idxs)

    kwargs = {}
    if _profile and _install_profile_shim():
        kwargs["trace"] = True
        if _trace_kwargs:
            kwargs["trace_kwargs"] = _trace_kwargs
    res = run_bass_kernel_spmd(nc, in_maps, list(range(N_CORES)), **kwargs)

    out = np.empty((b, h, n, d), np.float32)
    for c in range(N_CORES):
        o = res.results[c]["out"]  # [4, 128, 16, 64]
        bi = c // 2
        for hh in range(4):
            hi = (c % 2) * 4 + hh
            out[bi, hi] = o[hh].transpose(1, 0, 2).reshape(n, d)
    if _profile:
        return out, res
    return out


# revision 8
# speedup vs baseline: 1.4173x; 1.3382x over previous
"""Trainium2 Bass kernel for nn_Attend (l2-dist attention, b=4 h=8 n=2048 d=64).

Reference math:
    sim = 2*scale*(q@k^T) - ||q||^2 - ||k||^2   (scale = d^-0.5)
    sim = where(mask_j, sim, -FLT_MAX)
    out = softmax_j(sim) @ v

Device strategy (8 cores, pure data/head parallel, no collectives):
  - (b, h) pairs flattened; core c handles b = c//2, heads 4*(c%2)..+4.
  - ||q||^2 is constant per softmax row -> dropped (softmax shift-invariant).
  - mask is per (b, j): ~50% of keys masked.  Host compacts k/v to the valid
    columns only (padded to a multiple of 128), halving all device work.
  - No row-max pass: logits + C stay comfortably inside fp32 exp range.
    Per-key bias (C - ||k_j||^2, or -1e30 for padding) is applied via the
    ACT engine's per-partition bias during the exp.
  - Layout: S^T = K_c @ Q^T with keys on partitions (fp16 matmul, fp32 PSUM),
    exp on ACT -> P^T in fp16, then acc[i, 65] += P^T_slice^T @ [V|1] (fp16)
    accumulated over key tiles in PSUM.  Column 64 (ones) is the softmax
    denominator; DVE reciprocal + per-partition scale finishes the division.
  - q/k are duplicated into both partition halves so the two K=64 QK matmuls
    of a stage run CONCURRENTLY in different PE row-groups (row tiling);
    stages are software-pipelined so the next S^T computes during the
    current exp.  Host pre-transposes/casts/pads; device does no layout work.
"""

import os
import sys

import numpy as np

for _p in ("/root/.axon_site/_ro/trn_rl_repo", "/opt/trn_rl_repo"):
    if os.path.isdir(_p) and _p not in sys.path:
        sys.path.append(_p)

from contextlib import ExitStack

import concourse.bacc as bacc
import concourse.tile as tile
from concourse import mybir
from concourse.bass_utils import run_bass_kernel_spmd

N_CORES = 8
N_I = 2048          # queries per head
D = 64
HEADS_PER_CORE = 4
C_SHIFT = 30.0      # logit shift; keeps exp inputs in a comfortable range
PAD_BIAS = -1e30    # exp() underflows to exactly 0

_PROGRAM_CACHE = {}


def _build_program(j_tiles: int):
    """Bass program for one core: 4 heads of compacted attention."""
    nc = bacc.Bacc("TRN2", target_bir_lowering=False, debug=False)
    jp = j_tiles * 128
    f16, f32 = mybir.dt.float16, mybir.dt.float32

    # DRAM layouts mirror SBUF exactly: [128 partitions, ...] contiguous.
    qT = nc.dram_tensor("qT", [4, 128, N_I], f16, kind="ExternalInput").ap()
    kT = nc.dram_tensor("kT", [4, 128, jp], f16, kind="ExternalInput").ap()
    vS = nc.dram_tensor("vS", [4, 128, j_tiles * 65], f16, kind="ExternalInput").ap()
    bias = nc.dram_tensor("bias", [4, 128, j_tiles], f32, kind="ExternalInput").ap()
    out = nc.dram_tensor("out", [4, 128, 16, 64], f32, kind="ExternalOutput").ap()

    with tile.TileContext(nc) as tc, ExitStack() as ctx:
        inp = ctx.enter_context(tc.tile_pool(name="inp", bufs=1))
        pp = ctx.enter_context(tc.tile_pool(name="pp", bufs=3))
        outp = ctx.enter_context(tc.tile_pool(name="outp", bufs=2))
        rp = ctx.enter_context(tc.tile_pool(name="rp", bufs=2))
        ps_st = ctx.enter_context(tc.tile_pool(name="ps_st", bufs=2, space="PSUM"))
        ps_acc = ctx.enter_context(tc.tile_pool(name="ps_acc", bufs=4, space="PSUM"))

        # Per-head input tiles, DMA'd in head order so head 0 compute starts
        # as early as possible.
        qT_t, kT_t, vS_t, bias_t = [], [], [], []
        for hh in range(HEADS_PER_CORE):
            qt = inp.tile([128, N_I], f16, tag=f"q{hh}", name=f"qt{hh}")
            nc.sync.dma_start(qt[:], qT[hh])
            qT_t.append(qt)
            kt = inp.tile([128, jp], f16, tag=f"k{hh}", name=f"kt{hh}")
            nc.sync.dma_start(kt[:], kT[hh])
            kT_t.append(kt)
            bt = inp.tile([128, j_tiles], f32, tag=f"b{hh}", name=f"bt{hh}")
            nc.sync.dma_start(bt[:], bias[hh])
            bias_t.append(bt)
            vt = inp.tile([128, j_tiles * 65], f16, tag=f"v{hh}", name=f"vt{hh}")
            nc.sync.dma_start(vt[:], vS[hh])
            vS_t.append(vt)

        # Flat software pipeline over stages (hh, ih, jt): QK for stage s+1 is
        # emitted BEFORE the PV of stage s so the PE computes the next S^T
        # while ACT runs the current exp.
        stages = [
            (hh, ih, jt)
            for hh in range(HEADS_PER_CORE)
            for ih in range(2)
            for jt in range(j_tiles)
        ]
        st_tiles = {}
        acc_tiles = {}
        osb_tiles = {}

        def emit_qk(s):
            hh, ih, jt = stages[s]
            st = ps_st.tile([128, 1024], f32, tag="st", name=f"st_{hh}_{ih}_{jt}")
            # q/k live duplicated in both partition halves: the two K=64
            # matmuls go to PE row-groups 0 and 64 and run concurrently.
            for half in range(2):
                i0 = ih * 1024 + half * 512
                p0 = 64 * half
                nc.tensor.matmul(
                    st[:, half * 512:(half + 1) * 512],
                    kT_t[hh][p0:p0 + 64, jt * 128:(jt + 1) * 128],
                    qT_t[hh][p0:p0 + 64, i0:i0 + 512],
                    start=True, stop=True,
                )
            st_tiles[s] = st

        pt_tiles = {}

        def emit_pv(s):
            """PV + (at tile-row end) the divide/store drain for stage s."""
            hh, ih, jt = stages[s]
            pt = pt_tiles.pop(s)
            if jt == 0:
                acc_tiles[(hh, ih)] = [
                    ps_acc.tile([128, 4, 65], f32, tag="acc", name=f"acc_{hh}_{ih}_{g}")
                    for g in range(2)
                ]
            accs = acc_tiles[(hh, ih)]
            for sl in range(8):
                # start=True lazily zeroes the WHOLE 2KB psum bank (pending-
                # zero bits); only the first slice-matmul of each bank may
                # carry it.  Later slices at jt==0 then overwrite their
                # still-pending bytes.
                nc.tensor.matmul(
                    accs[sl // 4][:, sl % 4, :],
                    pt[:, sl * 128:(sl + 1) * 128],
                    vS_t[hh][:, jt * 65:(jt + 1) * 65],
                    start=(jt == 0 and sl % 4 == 0),
                    stop=(jt == j_tiles - 1 and sl % 4 == 3),
                    skip_group_check=True,
                )
            if jt == j_tiles - 1:
                if hh not in osb_tiles:
                    osb_tiles[hh] = outp.tile(
                        [128, 16, 64], f32, tag="osb", name=f"osb_{hh}"
                    )
                osb = osb_tiles[hh]
                for g in range(2):
                    r = rp.tile([128, 4], f32, tag="r", name=f"r_{hh}_{ih}_{g}")
                    nc.vector.reciprocal(r[:], accs[g][:, :, 64])
                    for sl in range(4):
                        t_idx = ih * 8 + g * 4 + sl
                        nc.vector.tensor_scalar_mul(
                            osb[:, t_idx, :], accs[g][:, sl, 0:64], r[:, sl:sl + 1]
                        )
                del acc_tiles[(hh, ih)]
                if ih == 1:
                    nc.sync.dma_start(out[hh], osb[:])
                    del osb_tiles[hh]

        # Emission per stage s: ACT(s); QK(s+1); PV(s-1).  In PE program
        # order QK(s+1) then runs at the START of the ACT(s) window (it only
        # waits for the st slot ACT(s-1) just released), so ACT(s+1) is never
        # gated on PE work; PV(s-1) fills the remaining PE time.
        emit_qk(0)
        for s, (hh, ih, jt) in enumerate(stages):
            st = st_tiles.pop(s)
            pt = pp.tile([128, 1024], f16, tag="pt", name=f"pt_{hh}_{ih}_{jt}")
            pt_tiles[s] = pt
            nc.scalar.activation(
                pt[:], st[:], mybir.ActivationFunctionType.Exp,
                bias=bias_t[hh][:, jt:jt + 1], scale=1.0,
            )
            if s + 1 < len(stages):
                emit_qk(s + 1)
            if s >= 1:
                emit_pv(s - 1)
        emit_pv(len(stages) - 1)

    nc.compile()
    return nc


def _get_program(j_tiles: int):
    if j_tiles not in _PROGRAM_CACHE:
        _PROGRAM_CACHE[j_tiles] = _build_program(j_tiles)
    return _PROGRAM_CACHE[j_tiles]


def _prepare_inputs(q, k, v, mask, j_tiles, idxs):
    """Host-side shard + compact + transpose + cast for each core."""
    b, h, n, d = q.shape
    scale = d ** -0.5
    jp = j_tiles * 128
    in_maps = []
    for c in range(N_CORES):
        bi = c // 2
        ix = idxs[bi]
        nv = len(ix)
        qT_np = np.zeros((4, 128, N_I), np.float16)
        kT_np = np.zeros((4, 128, jp), np.float16)
        vS_np = np.zeros((4, 128, j_tiles * 65), np.float16)
        bias_np = np.full((4, 128, j_tiles), PAD_BIAS, np.float32)
        for hh in range(4):
            hi = (c % 2) * 4 + hh
            qt = (2.0 * scale * q[bi, hi]).T.astype(np.float16)   # [64, 2048]
            qT_np[hh, 0:64, :] = qt
            qT_np[hh, 64:128, :] = qt
            kc = k[bi, hi, ix, :]
            kt = np.zeros((64, jp), np.float16)
            kt[:, :nv] = kc.T.astype(np.float16)
            kT_np[hh, 0:64, :] = kt
            kT_np[hh, 64:128, :] = kt
            vc = v[bi, hi, ix, :]
            va = np.concatenate(
                [vc, np.ones((nv, 1), np.float32)], axis=1
            ).astype(np.float16)
            vfull = np.zeros((jp, 65), np.float16)
            vfull[:nv] = va
            vS_np[hh] = (
                vfull.reshape(j_tiles, 128, 65).transpose(1, 0, 2)
                .reshape(128, j_tiles * 65)
            )
            ksq = (kc.astype(np.float64) ** 2).sum(-1).astype(np.float32)
            bfull = np.full((jp,), PAD_BIAS, np.float32)
            bfull[:nv] = C_SHIFT - ksq
            bias_np[hh] = bfull.reshape(j_tiles, 128).T
        in_maps.append({"qT": qT_np, "kT": kT_np, "vS": vS_np, "bias": bias_np})
    return in_maps


def _install_profile_shim():
    """Bridge concourse's NTFF trace path to the in-container profiler.

    concourse expects `antenv.axon_hooks.{get,set}_axon_ntff_profile_hook`;
    this image's antenv stub lacks it.  Recreate the module and register the
    ctypes hook from trn_agent_boot.  Also neuter upload_artifacts (no cloud
    bucket in-container).
    """
    import types

    try:
        import antenv
        if "antenv.axon_hooks" not in sys.modules:
            mod = types.ModuleType("antenv.axon_hooks")
            mod._hook = None

            def set_axon_ntff_profile_hook(h):
                mod._hook = h

            def get_axon_ntff_profile_hook():
                return mod._hook

            mod.set_axon_ntff_profile_hook = set_axon_ntff_profile_hook
            mod.get_axon_ntff_profile_hook = get_axon_ntff_profile_hook
            sys.modules["antenv.axon_hooks"] = mod
            antenv.axon_hooks = mod
        from antenv import axon_hooks
        if axon_hooks.get_axon_ntff_profile_hook() is None:
            from trn_agent_boot.trn_boot import _ntff_profile_via_ctypes
            axon_hooks.set_axon_ntff_profile_hook(
                _ntff_profile_via_ctypes("/opt/axon/libaxon_pjrt.so")
            )
        import concourse.bass_utils as bu
        bu.upload_artifacts = lambda d: str(d)
        return axon_hooks.get_axon_ntff_profile_hook() is not None
    except Exception as e:  # pragma: no cover - profiling is best-effort
        print(f"profile shim failed: {e}")
        return False


def kernel(q, k, v, mask, _profile=False, _trace_kwargs=None):
    q = np.asarray(q, dtype=np.float32)
    k = np.asarray(k, dtype=np.float32)
    v = np.asarray(v, dtype=np.float32)
    mask = np.asarray(mask)
    b, h, n, d = q.shape

    idxs = [np.nonzero(mask[bi])[0] for bi in range(b)]
    max_nv = max(max(len(ix) for ix in idxs), 1)
    j_tiles = -(-max_nv // 128)

    nc = _get_program(j_tiles)
    in_maps = _prepare_inputs(q, k, v, mask, j_tiles, idxs)

    kwargs = {}
    if _profile and _install_profile_shim():
        kwargs["trace"] = True
        if _trace_kwargs:
            kwargs["trace_kwargs"] = _trace_kwargs
    res = run_bass_kernel_spmd(nc, in_maps, list(range(N_CORES)), **kwargs)

    out = np.empty((b, h, n, d), np.float32)
    for c in range(N_CORES):
        o = res.results[c]["out"]  # [4, 128, 16, 64]
        bi = c // 2
        for hh in range(4):
            hi = (c % 2) * 4 + hh
            out[bi, hi] = o[hh].transpose(1, 0, 2).reshape(n, d)
    if _profile:
        return out, res
    return out
